# revision 16
# baseline (speedup 1.0000x reference)
"""Trainium2 Bass kernel for nn_Block_41893111005237 (Mamba2 + MQA + RWKV-CMix block).

Sharding: 2-way data-parallel over batch x 4-way tensor-parallel within each
group of 4 cores (mamba heads 8/core, attn q-heads 4/core with replicated KV,
FFN column/row split on W_key/W_val). Activations are feature-major [D, T]
on-chip. Large matmuls run as float32r (full PE rate at N=512); the SSD
chunked scan, attention probabilities, and the FFN use bf16 operands with
fp32 PSUM accumulation. ACT uses only exp/ln/abs/square/relu/identity (one
LUT table): softplus=ln(1+exp), sigmoid=exp(-ln(1+exp(-x))), rsqrt=exp(-.5 ln).
"""
import os
import sys
from contextlib import ExitStack

import numpy as np

for _p in ("/opt/trn_rl_repo", "/root/.axon_site/_ro/trn_rl_repo"):
    if os.path.isdir(_p) and _p not in sys.path:
        sys.path.insert(0, _p)

import ml_dtypes
import concourse.bass as bass
import concourse.tile as tile
from concourse import bacc, mybir
from concourse.bass import ts

f32 = mybir.dt.float32
f32r = mybir.dt.float32r
bf16 = mybir.dt.bfloat16
FT = mybir.ActivationFunctionType
OP = mybir.AluOpType

D = 1024
T = 1024
NCORES = 8
L = 128
NCH = 8
HPC = 8
P = 64
DI = 2048
AH = 4
HD = 64
EPS0 = 1e-6
EPS_G = 1e-5
GROUPS = [[0, 1, 2, 3], [4, 5, 6, 7]]

_CACHE = {}


def _patch_act_tables():
    # All ACT functions used here (exp/ln/copy/identity/square/abs/relu) live
    # in the natural_log_exp_and_others LUT set; restricting the chooser to it
    # avoids dozens of mid-kernel table reloads.
    import concourse.bacc as _bacc
    import concourse.hw_specs as _hw
    orig = _hw.get_activation_tables

    def only_lnexp(arch):
        t = orig(arch)
        sel = {k: v for k, v in t.items() if k == "natural_log_exp_and_others"}
        return sel or t

    _bacc.get_activation_tables = only_lnexp


def build_module():
    nc = bacc.Bacc("TRN2", target_bir_lowering=False, debug=False,
                   num_devices=NCORES)

    def din(name, shape, dt=f32r):
        return nc.dram_tensor(name, shape, dt, kind="ExternalInput").ap()

    xT_d = din("xT", [128, 8, T])
    wc_d = din("wc", [128, 8, 11, 128])
    wout_d = din("wout", [128, 4, 8, 128])
    wqkv_d = din("wqkv", [128, 8, 3, 128])
    wcp_d = din("wcp", [128, 2, 8, 128])
    wkey_d = din("wkey", [128, 8, 8, 128], bf16)
    wval_d = din("wval", [128, 8, 8, 128], bf16)
    wrec_d = din("wrec", [128, 8, 2, 128], bf16)
    cwm_d = din("cwm", [128, 6, 4], f32)
    cbm_d = din("cbm", [128, 6, 1], f32)
    cwa_d = din("cwa", [128, 3, 3], f32)
    cba_d = din("cba", [128, 3, 1], f32)
    acol_d = din("acol", [8, 1], f32)
    dtb_d = din("dtb", [8, 1], f32)
    dmrep_d = din("dmrep", [128, 4, 1], f32)
    maak_d = din("maak", [128, 8, 1], f32)
    maak1_d = din("maak1", [128, 8, 1], f32)
    maar_d = din("maar", [128, 8, 1], f32)
    maar1_d = din("maar1", [128, 8, 1], f32)
    idr_d = din("idr", [128, 128], f32r)
    idf_d = din("idf", [128, 128], f32)
    onesr_d = din("onesr", [128, 1], f32r)
    onesrow_d = din("onesrow", [1, 128], f32r)
    onesb_d = din("onesb", [128, 1], bf16)
    maskg_d = din("maskg", [128, 128], f32)
    maska_d = din("maska", [128, 896], bf16)

    out_gkv = nc.dram_tensor("out_gkv", [128, 2, T], f32,
                             kind="ExternalOutput").ap()
    out_x2 = nc.dram_tensor("out_x2", [128, 8, T], f32,
                            kind="ExternalOutput").ap()

    with tile.TileContext(nc) as tc, ExitStack() as ctx:
        outer = ctx.enter_context(tc.tile_pool(name="outer", bufs=1))
        scr = ctx.enter_context(tc.tile_pool(name="scr", bufs=2))
        wpool = ctx.enter_context(tc.tile_pool(name="wmt", bufs=4))
        dram = ctx.enter_context(tc.tile_pool(name="dram", bufs=1, space="DRAM"))

        def cinit(name, dram_ap, shape, dt=f32):
            t = outer.tile(shape, dt, tag=name)
            nc.sync.dma_start(t[:], dram_ap)
            return t

        idr = cinit("idr", idr_d, [128, 128], f32r)
        idf = cinit("idf", idf_d, [128, 128])
        onesr = cinit("onesr", onesr_d, [128, 1], f32r)
        onesrow = cinit("onesrow", onesrow_d, [1, 128], f32r)
        onesb = cinit("onesb", onesb_d, [128, 1], bf16)
        maskg = cinit("maskg", maskg_d, [128, 128])
        maska = cinit("maska", maska_d, [128, 896], bf16)
        cwm = cinit("cwm", cwm_d, [128, 6, 4])
        cbm = cinit("cbm", cbm_d, [128, 6, 1])
        cwa = cinit("cwa", cwa_d, [128, 3, 3])
        cba = cinit("cba", cba_d, [128, 3, 1])
        acol = cinit("acol", acol_d, [8, 1])
        dtb = cinit("dtb", dtb_d, [8, 1])
        dmrep = cinit("dmrep", dmrep_d, [128, 4, 1])
        maak = cinit("maak", maak_d, [128, 8, 1])
        maak1 = cinit("maak1", maak1_d, [128, 8, 1])
        maar = cinit("maar", maar_d, [128, 8, 1])
        maar1 = cinit("maar1", maar1_d, [128, 8, 1])
        epsrow = outer.tile([1, 1], f32, tag="epsrow")

        xres = outer.tile([128, 8, T], f32r, tag="xres")
        nc.sync.dma_start(xres[:], xT_d)

        def make_rs(get_kt, n_kt, den, eps_val, label, ps):
            acc = [ps.tile([1, 512], f32, tag="ssq", name=f"ssqa{i}") for i in range(2)]
            for kt in range(n_kt):
                for tb in range(2):
                    sq = scr.tile([128, 512], f32r, tag="sqws")
                    nc.scalar.activation(sq[:], get_kt(kt)[:, ts(tb, 512)],
                                         FT.Square)
                    nc.tensor.matmul(acc[tb][:], onesr[:], sq[:],
                                     start=(kt == 0), stop=(kt == n_kt - 1))
            nc.vector.memset(epsrow[:], eps_val)
            lnrow = scr.tile([1, T], f32, tag="lnrow", bufs=1)
            for tb in range(2):
                nc.scalar.activation(lnrow[:, ts(tb, 512)], acc[tb][:],
                                     FT.Ln, bias=epsrow[:], scale=1.0 / den)
            rsrow = scr.tile([1, T], f32r, tag="rsrow", bufs=1)
            nc.scalar.activation(rsrow[:], lnrow[:], FT.Exp, scale=-0.5)
            return rsrow

        def bcast_row(rsrow, ps):
            out_sb = scr.tile([128, T], f32, tag="rbX", bufs=1)
            for tb in range(2):
                pt = ps.tile([128, 512], f32, tag="mm")
                nc.tensor.matmul(pt[:], onesrow[:], rsrow[:, ts(tb, 512)],
                                 start=True, stop=True)
                nc.scalar.copy(out_sb[:, ts(tb, 512)], pt[:])
            return out_sb

        def psum_to_dram(pt_ap, dram_ap):
            stg = scr.tile([128, 512], f32, tag="stg", name="stg")
            nc.scalar.copy(stg[:], pt_ap)
            nc.sync.dma_start(dram_ap, stg[:])

        def sigmoid_into(out_ap, x_ap, pool, inplace_x=False):
            # out = sigmoid(x) = exp(-ln(1+exp(-x)))
            e = out_ap if inplace_x else pool.tile([128, x_ap.shape[-1]], f32,
                                                   tag="sigt")
            nc.scalar.activation(e[:], x_ap, FT.Exp, scale=-1.0)
            nc.vector.tensor_scalar_add(e[:], e[:], 1.0)
            nc.scalar.activation(e[:], e[:], FT.Ln)
            nc.scalar.activation(out_ap, e[:], FT.Exp, scale=-1.0)

        def silu_into(out_ap, x_ap, pool):
            s = pool.tile([128, x_ap.shape[-1]], f32, tag="silt")
            sigmoid_into(s[:], x_ap, pool, inplace_x=True)
            nc.vector.tensor_tensor(out_ap, x_ap, s[:], OP.mult)

        # ================= Stage M =================
        with tc.tile_pool(name="stM", bufs=1) as sm, \
             tc.tile_pool(name="stM2", bufs=2) as sm2, \
             tc.tile_pool(name="pmm", bufs=3, space="PSUM") as pmm, \
             tc.tile_pool(name="pssq", bufs=2, space="PSUM") as pssq, \
             tc.tile_pool(name="psp", bufs=3, space="PSUM") as psp:

            rs0 = make_rs(lambda kt: xres[:, kt, :].bitcast(f32), 8, D, EPS0,
                          "0", pssq)
            rs0b = bcast_row(rs0, pmm)

            xbcp = sm.tile([128, 6, T + 3], f32r, tag="conv")
            nc.vector.memset(xbcp[:, :, 0:3].bitcast(f32), 0.0)
            dtpre = sm.tile([8, T], f32, tag="dtpre")
            sz = sm.tile([128, 4, T], bf16, tag="sz")

            for mt in [10] + list(range(10)):
                wt = wpool.tile([128, 8, 128], f32r, tag="wmt", bufs=3)
                nc.sync.dma_start(wt[:], wc_d[:, :, mt, :])
                for tb in range(2):
                    pt = pmm.tile([128, 512], f32, tag="mm")
                    for kt in range(8):
                        nc.tensor.matmul(pt[:], wt[:, kt, :],
                                         xres[:, kt, ts(tb, 512)],
                                         start=(kt == 0), stop=(kt == 7))
                    if mt == 10:
                        nc.vector.tensor_tensor(dtpre[:, ts(tb, 512)],
                                                pt[0:8, :], rs0b[0:8, ts(tb, 512)],
                                                OP.mult)
                    elif mt < 4:
                        zt = sm2.tile([128, 512], f32, tag="ztmp")
                        nc.vector.tensor_tensor(zt[:], pt[:], rs0b[:, ts(tb, 512)],
                                                OP.mult)
                        silu_into(sz[:, mt, ts(tb, 512)], zt[:], sm2)
                    else:
                        nc.vector.tensor_tensor(
                            xbcp[:, mt - 4, 3 + tb * 512:3 + (tb + 1) * 512],
                            pt[:], rs0b[:, ts(tb, 512)], OP.mult)

            for i in range(6):
                tmpc = sm2.tile([128, T], f32, tag="convtmp")
                nc.scalar.activation(tmpc[:], xbcp[:, i, 0:T].bitcast(f32),
                                     FT.Identity,
                                     bias=cbm[:, i, :], scale=cwm[:, i, 0:1])
                for k in range(1, 4):
                    nc.vector.scalar_tensor_tensor(
                        tmpc[:], xbcp[:, i, k:k + T].bitcast(f32),
                        cwm[:, i, k:k + 1], tmpc[:], OP.mult, OP.add)
                silu_into(xbcp[:, i, 3:3 + T], tmpc[:], sm2)
            convo = xbcp[:, :, 3:3 + T]

            # dt -> softplus; chunk-local cumsums
            dtsp = sm.tile([8, T], f32, tag="dtsp")
            nc.scalar.activation(dtsp[:], dtpre[:], FT.Exp, bias=dtb[:])
            nc.vector.tensor_scalar_add(dtsp[:], dtsp[:], 1.0)
            nc.scalar.activation(dtsp[:], dtsp[:], FT.Ln)
            dtA = sm.tile([8, T], f32, tag="dtA")
            nc.vector.tensor_scalar_mul(dtA[:], dtsp[:], acol[:])
            zr8 = sm.tile([8, L], f32, tag="zr8")
            nc.vector.memset(zr8[:], 0.0)
            lcs = sm.tile([8, NCH, L], f32, tag="lcs")
            for c in range(NCH):
                nc.vector.tensor_tensor_scan(lcs[:, c, :], dtA[:, ts(c, L)],
                                             zr8[:], 0.0, OP.add, OP.add)
            lend0 = sm.tile([1, 8, 8], f32, tag="lend0")
            nc.sync.dma_start(lend0[:], lcs[:, :, L - 1])
            lts = sm.tile([128, 8, 8], f32, tag="lts")
            dtspT = sm.tile([128, 8, 8], f32, tag="dtspT")
            for c in range(NCH):
                ptr = psp.tile([128, 512], f32, tag="sp")
                nc.tensor.transpose(ptr[0:128, 0:8], lcs[:, c, :], idf[0:8, 0:8])
                nc.scalar.copy(lts[:, :, c], ptr[0:128, 0:8])
                ptr2 = psp.tile([128, 512], f32, tag="sp")
                nc.tensor.transpose(ptr2[0:128, 0:8], dtsp[:, ts(c, L)],
                                    idf[0:8, 0:8])
                nc.scalar.copy(dtspT[:, :, c], ptr2[0:128, 0:8])
            lrep = sm.tile([128, 8, 8], f32, tag="lrep")
            for h in range(HPC):
                nc.gpsimd.partition_broadcast(lrep[:, h, :], lend0[:, h, :])
            elrep = sm.tile([128, 8, 8], f32, tag="elrep")
            nc.scalar.activation(elrep[:], lrep[:], FT.Exp)
            fc = sm.tile([128, 8, 8], f32, tag="fc")
            nc.vector.tensor_tensor(fc[:], lrep[:], lts[:], OP.subtract)
            nc.scalar.activation(fc[:], fc[:], FT.Exp)
            nc.vector.tensor_tensor(fc[:], fc[:], dtspT[:], OP.mult)

            ym = sm.tile([128, 4, T], f32r, tag="ym")
            hst = [sm.tile([128, HPC, P], bf16, tag=f"hst{i}", name=f"hst{i}") for i in range(2)]
            nc.vector.memset(hst[0][:].bitcast(mybir.dt.uint16), 0)

            for c in range(NCH):
                csl = ts(c, L)
                gp = psp.tile([128, 512], f32, tag="sp")
                nc.tensor.matmul(gp[0:128, 0:128], convo[:, 4, csl],
                                 convo[:, 5, csl], start=True, stop=True)
                gm = sm2.tile([128, 128], f32, tag="gm")
                nc.vector.tensor_tensor(gm[:], gp[0:128, 0:128], maskg[:], OP.mult)
                btp = psp.tile([128, 512], f32r, tag="sp")
                nc.tensor.transpose(btp[0:128, 0:128], convo[:, 4, csl], idr[:])
                btm = sm2.tile([128, 128], bf16, tag="btm")
                nc.scalar.copy(btm[:], btp[0:128, 0:128].bitcast(f32))
                xtm = sm2.tile([128, HPC, P], bf16, tag="xtm")
                for pr in range(4):
                    xp = psp.tile([128, 512], f32r, tag="sp")
                    nc.tensor.transpose(xp[0:128, 0:128], convo[:, pr, csl], idr[:])
                    nc.scalar.copy(
                        xtm[:, pr * 2:pr * 2 + 2, :],
                        xp[0:128, 0:128].bitcast(f32)
                        .rearrange("p (a b) -> p a b", a=2))
                lcs0c = sm2.tile([1, 8, L], f32, tag="lcs0c")
                nc.sync.dma_start(lcs0c[:], lcs[:, c, :])
                lball = sm2.tile([128, HPC, L], f32, tag="lball", bufs=1)
                for h in range(HPC):
                    nc.gpsimd.partition_broadcast(lball[:, h, :],
                                                  lcs0c[:, h, :])
                mall = sm2.tile([128, HPC, L], f32, tag="mall", bufs=1)
                nc.vector.tensor_tensor(
                    mall[:], lball[:],
                    lts[:, :, c:c + 1].broadcast_to((128, 8, L)), OP.subtract)
                nc.vector.tensor_scalar_min(mall[:], mall[:], 0.0)
                nc.scalar.activation(mall[:], mall[:], FT.Exp)
                eall = sm2.tile([128, HPC, L], bf16, tag="eall")
                nc.scalar.activation(eall[:], lball[:], FT.Exp)
                sall = sm2.tile([128, HPC, L], bf16, tag="sall")
                nc.vector.tensor_tensor(
                    sall[:],
                    gm[:].rearrange("p (o t) -> p o t", o=1).broadcast_to((128, 8, L)),
                    mall[:], OP.mult)
                chat = sm2.tile([128, HPC, L], bf16, tag="chat")
                nc.vector.tensor_tensor(
                    chat[:],
                    convo[:, 5, csl].bitcast(f32)
                    .rearrange("p (o t) -> p o t", o=1)
                    .broadcast_to((128, 8, L)),
                    eall[:], OP.mult)
                dtx = sm2.tile([128, HPC, P], bf16, tag="dtx")
                nc.vector.tensor_tensor(
                    dtx[:], xtm[:],
                    dtspT[:, :, c:c + 1].broadcast_to((128, 8, P)), OP.mult)
                dtxd = sm2.tile([128, HPC, P], bf16, tag="dtxd")
                nc.vector.tensor_tensor(
                    dtxd[:], xtm[:],
                    fc[:, :, c:c + 1].broadcast_to((128, 8, P)), OP.mult)
                hprev, hnew = hst[c % 2], hst[(c + 1) % 2]
                updp = psp.tile([128, 512], f32, tag="sp")
                for h in range(HPC):
                    ypp = pmm.tile([128, 512], f32, tag="mm")
                    nc.tensor.matmul(ypp[0:P, 0:L], dtx[:, h, :], sall[:, h, :],
                                     start=True, stop=False)
                    nc.tensor.matmul(ypp[0:P, 0:L], hprev[:, h, :], chat[:, h, :],
                                     start=False, stop=True)
                    r0 = (h % 2) * P
                    nc.scalar.copy(ym[r0:r0 + P, h // 2, csl], ypp[0:P, 0:L])
                    nc.tensor.matmul(updp[:, ts(h, P)], btm[:], dtxd[:, h, :],
                                     start=True, stop=True)
                nc.vector.tensor_tensor(
                    hnew[:], hprev[:],
                    elrep[:, :, c:c + 1].broadcast_to((128, 8, P)), OP.mult)
                nc.vector.tensor_tensor(
                    hnew[:], hnew[:],
                    updp[:].rearrange("p (h q) -> p h q", h=8), OP.add)

            for i in range(4):
                nc.vector.scalar_tensor_tensor(
                    ym[:, i, :], convo[:, i, :].bitcast(f32), dmrep[:, i, :],
                    ym[:, i, :].bitcast(f32), OP.mult, OP.add)
                nc.vector.tensor_tensor(ym[:, i, :], ym[:, i, :].bitcast(f32),
                                        sz[:, i, :], OP.mult)

            ar1_in = dram.tile([1025, T], f32)
            sqacc = [pssq.tile([1, 512], f32, tag="ssq", name=f"ssqb{i}") for i in range(2)]
            for i in range(4):
                sq = scr.tile([128, T], f32r, tag="sqws")
                nc.scalar.activation(sq[:], ym[:, i, :].bitcast(f32), FT.Square)
                for tb in range(2):
                    nc.tensor.matmul(sqacc[tb][:], onesr[:], sq[:, ts(tb, 512)],
                                     start=(i == 0), stop=(i == 3))
            sqrow = scr.tile([1, T], f32, tag="lnrow", name="sqrow", bufs=1)
            for tb in range(2):
                nc.scalar.copy(sqrow[:, ts(tb, 512)], sqacc[tb][:])
            nc.sync.dma_start(ar1_in[1024:1025, :], sqrow[:])
            for mt in range(8):
                wo = wpool.tile([128, 8, 128], f32r, tag="wmt", bufs=3)
                nc.sync.dma_start(wo[:, 0:4, :], wout_d[:, :, mt, :])
                for tb in range(2):
                    pt = pmm.tile([128, 512], f32, tag="mm")
                    for kt in range(4):
                        nc.tensor.matmul(pt[:], wo[:, kt, :],
                                         ym[:, kt, ts(tb, 512)],
                                         start=(kt == 0), stop=(kt == 3))
                    psum_to_dram(pt[:], ar1_in[ts(mt, 128), ts(tb, 512)])

            ar1_out = dram.tile([1025, T], f32)
            nc.gpsimd.collective_compute(
                "AllReduce", OP.add, replica_groups=GROUPS,
                ins=[ar1_in.opt()], outs=[ar1_out.opt()])

            gs = scr.tile([1, T], f32, tag="lnrow", name="gs", bufs=1)
            nc.sync.dma_start(gs[:], ar1_out[1024:1025, :])
            nc.vector.memset(epsrow[:], EPS_G)
            nc.scalar.activation(gs[:], gs[:], FT.Ln, bias=epsrow[:],
                                 scale=1.0 / DI)
            rsg = scr.tile([1, T], f32r, tag="rsrow", name="rsg", bufs=1)
            nc.scalar.activation(rsg[:], gs[:], FT.Exp, scale=-0.5)
            rsgb = bcast_row(rsg, pmm)
            for kt in range(8):
                smt = scr.tile([128, T], f32, tag="sumt", bufs=1)
                nc.sync.dma_start(smt[:], ar1_out[ts(kt, 128), :])
                nc.vector.tensor_tensor(smt[:], smt[:], rsgb[:], OP.mult)
                nc.vector.tensor_tensor(xres[:, kt, :],
                                        xres[:, kt, :].bitcast(f32), smt[:],
                                        OP.add)

        # ================= Stage A =================
        with tc.tile_pool(name="stA", bufs=1) as sa, \
             tc.tile_pool(name="stA2", bufs=2) as sa2, \
             tc.tile_pool(name="stA3", bufs=3) as sa3, \
             tc.tile_pool(name="pmm", bufs=3, space="PSUM") as pmm, \
             tc.tile_pool(name="pssq", bufs=2, space="PSUM") as pssq, \
             tc.tile_pool(name="psp", bufs=2, space="PSUM") as psp:

            rs1 = make_rs(lambda kt: xres[:, kt, :].bitcast(f32), 8, D, EPS0,
                          "1", pssq)
            rs1b = bcast_row(rs1, pmm)
            qkvs = sa.tile([128, 3, T + 2], f32, tag="qkvs")
            nc.vector.memset(qkvs[:, :, 0:2], 0.0)
            for mt in range(3):
                wq = wpool.tile([128, 8, 128], f32r, tag="wmt", bufs=3)
                nc.sync.dma_start(wq[:], wqkv_d[:, :, mt, :])
                for tb in range(2):
                    pt = pmm.tile([128, 512], f32, tag="mm")
                    for kt in range(8):
                        nc.tensor.matmul(pt[:], wq[:, kt, :],
                                         xres[:, kt, ts(tb, 512)],
                                         start=(kt == 0), stop=(kt == 7))
                    nc.vector.tensor_tensor(
                        qkvs[:, mt, 2 + tb * 512:2 + (tb + 1) * 512], pt[:],
                        rs1b[:, ts(tb, 512)], OP.mult)
            convA = sa.tile([128, 3, T], f32r, tag="convA")
            for i in range(3):
                tmpc = sa2.tile([128, T], f32, tag="convtA")
                nc.scalar.activation(tmpc[:], qkvs[:, i, 0:T], FT.Identity,
                                     bias=cba[:, i, :], scale=cwa[:, i, 0:1])
                for k in range(1, 3):
                    nc.vector.scalar_tensor_tensor(
                        tmpc[:], qkvs[:, i, k:k + T], cwa[:, i, k:k + 1],
                        tmpc[:], OP.mult, OP.add)
                nc.vector.tensor_copy(convA[:, i, :], tmpc[:])
            ka2 = sa.tile([128, T], f32r, tag="ka2")
            nc.sync.dma_start(ka2[0:64, :], convA[0:64, 2, :])
            nc.sync.dma_start(ka2[64:128, :], convA[0:64, 2, :])
            vtm = sa.tile([128, NCH, HD + 1], bf16, tag="vtm")
            nc.vector.tensor_copy(
                vtm[:, :, HD:HD + 1],
                onesb[:].rearrange("p (a o) -> p a o", a=1).broadcast_to((128, 8, 1)))
            for tk in range(NCH):
                vp = psp.tile([128, 512], f32r, tag="sp")
                nc.tensor.transpose(vp[0:128, 0:HD], convA[64:128, 2, ts(tk, L)],
                                    idr[64:128, 64:128])
                nc.scalar.copy(vtm[:, tk, 0:HD], vp[0:128, 0:HD].bitcast(f32))
            yat = sa.tile([128, 2, T], f32r, tag="yat")
            for h in range(AH):
                q0 = (h % 2) * 64
                for tb in range(2):
                    ypp = pmm.tile([128, 512], f32, tag="mm")
                    ntk = 4 * (tb + 1)
                    for tk in range(ntk):
                        sp = pmm.tile([128, 512], f32, tag="mm")
                        nc.tensor.matmul(
                            sp[:], ka2[q0:q0 + 64, ts(tk, L)],
                            convA[q0:q0 + 64, h // 2, ts(tb, 512)],
                            start=True, stop=True)
                        ptile = sa3.tile([128, 512], bf16, tag="ptile")
                        nc.scalar.activation(ptile[:], sp[:], FT.Exp, scale=0.125)
                        delta = tb * 512 - tk * 128
                        if delta < 127:
                            nc.vector.tensor_tensor(
                                ptile[:], ptile[:],
                                maska[:, 384 + delta:896 + delta], OP.mult)
                        nc.tensor.matmul(ypp[0:HD + 1, :], vtm[:, tk, :],
                                         ptile[:], start=(tk == 0),
                                         stop=(tk == ntk - 1))
                    denr = sa2.tile([1, 512], f32, tag="denr")
                    nc.scalar.activation(denr[:], ypp[HD:HD + 1, :], FT.Ln)
                    rd = sa2.tile([1, 512], f32r, tag="rd")
                    nc.scalar.activation(rd[:], denr[:], FT.Exp, scale=-1.0)
                    rdp = psp.tile([128, 512], f32, tag="sp")
                    nc.tensor.matmul(rdp[0:64, :], onesrow[:, 0:64], rd[:],
                                     start=True, stop=True)
                    rdb = sa2.tile([64, 512], f32, tag="rdb")
                    nc.scalar.copy(rdb[:], rdp[0:64, :])
                    nc.vector.tensor_tensor(
                        yat[q0:q0 + 64, h // 2, ts(tb, 512)],
                        ypp[0:HD, :], rdb[:], OP.mult)
            ar2_in = dram.tile([1024, T], f32)
            for mt in range(8):
                wcpt = wpool.tile([128, 8, 128], f32r, tag="wmt", bufs=3)
                nc.sync.dma_start(wcpt[:, 0:2, :], wcp_d[:, :, mt, :])
                for tb in range(2):
                    pt = pmm.tile([128, 512], f32, tag="mm")
                    for kt in range(2):
                        nc.tensor.matmul(pt[:], wcpt[:, kt, :],
                                         yat[:, kt, ts(tb, 512)],
                                         start=(kt == 0), stop=(kt == 1))
                    psum_to_dram(pt[:], ar2_in[ts(mt, 128), ts(tb, 512)])
            ar2_out = dram.tile([1024, T], f32)
            nc.gpsimd.collective_compute(
                "AllReduce", OP.add, replica_groups=GROUPS,
                ins=[ar2_in.opt()], outs=[ar2_out.opt()])
            for kt in range(8):
                smt = scr.tile([128, T], f32, tag="sumt", bufs=1)
                nc.sync.dma_start(smt[:], ar2_out[ts(kt, 128), :])
                nc.vector.tensor_tensor(xres[:, kt, :],
                                        xres[:, kt, :].bitcast(f32), smt[:],
                                        OP.add)
            nc.sync.dma_start(out_x2, xres[:].bitcast(f32))

        # ================= Stage F =================
        with tc.tile_pool(name="stF", bufs=1) as sf, \
             tc.tile_pool(name="stF2", bufs=2) as sf2, \
             tc.tile_pool(name="pmm", bufs=3, space="PSUM") as pmm, \
             tc.tile_pool(name="pssq", bufs=2, space="PSUM") as pssq:

            rs2 = make_rs(lambda kt: xres[:, kt, :].bitcast(f32), 8, D, EPS0,
                          "2", pssq)
            rs2b = bcast_row(rs2, pmm)
            h2 = sf.tile([128, 8, T + 1], f32, tag="h2")
            nc.vector.memset(h2[:, :, 0:1], 0.0)
            for kt in range(8):
                nc.vector.tensor_tensor(h2[:, kt, 1:T + 1],
                                        xres[:, kt, :].bitcast(f32), rs2b[:],
                                        OP.mult)
            xk = sf.tile([128, 8, T], bf16, tag="xk")
            xr = sf.tile([128, 8, T], bf16, tag="xr")
            for kt in range(8):
                xsc = sf2.tile([128, T], f32, tag="xscr")
                nc.vector.tensor_scalar_mul(xsc[:], h2[:, kt, 1:T + 1],
                                            maak1[:, kt, :])
                nc.vector.scalar_tensor_tensor(
                    xk[:, kt, :], h2[:, kt, 0:T], maak[:, kt, :], xsc[:],
                    OP.mult, OP.add)
                xsc2 = sf2.tile([128, T], f32, tag="xscr")
                nc.vector.tensor_scalar_mul(xsc2[:], h2[:, kt, 1:T + 1],
                                            maar1[:, kt, :])
                nc.vector.scalar_tensor_tensor(
                    xr[:, kt, :], h2[:, kt, 0:T], maar[:, kt, :], xsc2[:],
                    OP.mult, OP.add)
            kf = sf.tile([128, 8, T], bf16, tag="kf")
            for mt in range(8):
                wk = wpool.tile([128, 8, 128], bf16, tag="wmtb", bufs=2)
                nc.sync.dma_start(wk[:], wkey_d[:, :, mt, :])
                for tb in range(2):
                    pt = pmm.tile([128, 512], f32, tag="mm")
                    for kt in range(8):
                        nc.tensor.matmul(pt[:], wk[:, kt, :],
                                         xk[:, kt, ts(tb, 512)],
                                         start=(kt == 0), stop=(kt == 7))
                    rl = sf2.tile([128, 512], f32, tag="rl")
                    nc.scalar.activation(rl[:], pt[:], FT.Relu)
                    nc.scalar.activation(kf[:, mt, ts(tb, 512)], rl[:],
                                         FT.Square)
            rsc_in = dram.tile([1024, T], f32)
            for mt in range(8):
                wv = wpool.tile([128, 8, 128], bf16, tag="wmtb", bufs=2)
                nc.sync.dma_start(wv[:], wval_d[:, :, mt, :])
                for tb in range(2):
                    pt = pmm.tile([128, 512], f32, tag="mm")
                    for kt in range(8):
                        nc.tensor.matmul(pt[:], wv[:, kt, :],
                                         kf[:, kt, ts(tb, 512)],
                                         start=(kt == 0), stop=(kt == 7))
                    psum_to_dram(pt[:], rsc_in[ts(mt, 128), ts(tb, 512)])
            rsc_out = dram.tile([256, T], f32)
            nc.gpsimd.collective_compute(
                "ReduceScatter", OP.add, replica_groups=GROUPS,
                ins=[rsc_in.opt()], outs=[rsc_out.opt()])
            kvr = sf.tile([128, 2, T], f32, tag="kvr")
            nc.sync.dma_start(kvr[:],
                              rsc_out[:].rearrange("(k p) t -> p k t", p=128))
            sg = sf.tile([128, 2, T], f32, tag="sg")
            for mt in range(2):
                wr = wpool.tile([128, 8, 128], bf16, tag="wmtb", bufs=2)
                nc.sync.dma_start(wr[:], wrec_d[:, :, mt, :])
                for tb in range(2):
                    pt = pmm.tile([128, 512], f32, tag="mm")
                    for kt in range(8):
                        nc.tensor.matmul(pt[:], wr[:, kt, :],
                                         xr[:, kt, ts(tb, 512)],
                                         start=(kt == 0), stop=(kt == 7))
                    e_ = sf2.tile([128, 512], f32, tag="sige")
                    nc.scalar.activation(e_[:], pt[:], FT.Exp, scale=-1.0)
                    nc.vector.tensor_scalar_add(e_[:], e_[:], 1.0)
                    nc.scalar.activation(e_[:], e_[:], FT.Ln)
                    nc.scalar.activation(sg[:, mt, ts(tb, 512)], e_[:], FT.Exp,
                                         scale=-1.0)
            nc.vector.tensor_tensor(sg[:], sg[:], kvr[:], OP.mult)
            nc.sync.dma_start(out_gkv, sg[:])

    nc.compile()
    return nc


def _w_tiles(w, kt, mt, dt=np.float32):
    Dk_, Mm_ = kt * 128, mt * 128
    assert w.shape == (Dk_, Mm_), (w.shape, kt, mt)
    return np.ascontiguousarray(
        w.reshape(kt, 128, mt, 128).transpose(1, 0, 2, 3)).astype(dt)


def make_in_maps(inputs):
    f = lambda k: np.asarray(inputs[k], np.float32)
    x = f("x")
    W_in = f("W_in"); conv_w = f("conv_w"); conv_b = f("conv_b")
    A = -np.exp(f("A_log")); Dm = f("Dm"); dtbv = f("dt_bias")
    W_out = f("W_out") * f("mnorm_w")[:, None]
    W_qkv = f("W_qkv"); W_cproj = f("W_cproj")
    qw, qb = f("qconv_w"), f("qconv_b")
    kw, kb = f("kconv_w"), f("kconv_b")
    vw, vb = f("vconv_w"), f("vconv_b")
    maa_k = f("time_maa_k"); maa_r = f("time_maa_r")
    W_key = f("W_key"); W_rec = f("W_rec"); W_val = f("W_val")
    bfdt = ml_dtypes.bfloat16

    idm = np.eye(128, dtype=np.float32)
    maskg = (np.arange(128)[:, None] <= np.arange(128)[None, :]).astype(np.float32)
    cgrid = np.arange(896)[None, :] - 384
    maska = (np.arange(128)[:, None] <= cgrid).astype(bfdt)
    cwa_full = np.concatenate([qw, qw, qw, qw, kw, vw], 0)       # (384, 3)
    cba_full = np.concatenate([qb, qb, qb, qb, kb, vb], 0)

    in_maps = []
    for core in range(NCORES):
        b, g = core // 4, core % 4
        zc = W_in[:, g * 512:(g + 1) * 512]
        xc = W_in[:, 2048 + g * 512:2048 + (g + 1) * 512]
        Bc = W_in[:, 4096:4224]; Cc = W_in[:, 4224:4352]
        dc = W_in[:, 4352 + g * 8:4352 + (g + 1) * 8]
        dpad = np.zeros((D, 120), np.float32)
        W_core = np.concatenate([zc, xc, Bc, Cc, dc, dpad], 1)
        cw = np.concatenate([conv_w[g * 512:(g + 1) * 512], conv_w[2048:2304]], 0)
        cb = np.concatenate([conv_b[g * 512:(g + 1) * 512], conv_b[2048:2304]], 0)
        Wq_c = np.concatenate([W_qkv[:, g * 256:(g + 1) * 256],
                               W_qkv[:, 1024:1152]], 1)
        m = {
            "xT": np.ascontiguousarray(
                x[b].T.reshape(8, 128, T).transpose(1, 0, 2)),
            "wc": _w_tiles(W_core, 8, 11),
            "wout": _w_tiles(W_out[g * 512:(g + 1) * 512], 4, 8),
            "wqkv": _w_tiles(Wq_c, 8, 3),
            "wcp": _w_tiles(W_cproj[g * 256:(g + 1) * 256], 2, 8),
            "wkey": _w_tiles(W_key[:, g * 1024:(g + 1) * 1024], 8, 8, bfdt),
            "wval": _w_tiles(W_val[g * 1024:(g + 1) * 1024], 8, 8, bfdt),
            "wrec": _w_tiles(W_rec[:, g * 256:(g + 1) * 256], 8, 2, bfdt),
            "cwm": np.ascontiguousarray(cw.reshape(6, 128, 4).transpose(1, 0, 2)),
            "cbm": np.ascontiguousarray(cb.reshape(6, 128, 1).transpose(1, 0, 2)),
            "cwa": np.ascontiguousarray(
                cwa_full.reshape(3, 128, 3).transpose(1, 0, 2)),
            "cba": np.ascontiguousarray(
                cba_full.reshape(3, 128, 1).transpose(1, 0, 2)),
            "acol": A[g * 8:(g + 1) * 8, None],
            "dtb": dtbv[g * 8:(g + 1) * 8, None],
            "dmrep": np.ascontiguousarray(
                np.repeat(Dm[g * 8:(g + 1) * 8], 64)
                .reshape(4, 128, 1).transpose(1, 0, 2)),
            "maak": np.ascontiguousarray(
                maa_k.reshape(8, 128, 1).transpose(1, 0, 2)),
            "maak1": np.ascontiguousarray(
                (1.0 - maa_k).reshape(8, 128, 1).transpose(1, 0, 2)),
            "maar": np.ascontiguousarray(
                maa_r.reshape(8, 128, 1).transpose(1, 0, 2)),
            "maar1": np.ascontiguousarray(
                (1.0 - maa_r).reshape(8, 128, 1).transpose(1, 0, 2)),
            "idr": idm, "idf": idm,
            "onesr": np.ones((128, 1), np.float32),
            "onesrow": np.ones((1, 128), np.float32),
            "onesb": np.ones((128, 1), bfdt),
            "maskg": maskg, "maska": maska,
        }
        out = {}
        for k, v in m.items():
            if v.dtype == bfdt:
                out[k] = np.ascontiguousarray(v)
            else:
                out[k] = np.ascontiguousarray(v, np.float32)
        in_maps.append(out)
    return in_maps


def assemble(results):
    out = np.zeros((2, T, D), np.float32)
    for core in range(NCORES):
        b, g = core // 4, core % 4
        r = results[core]
        x2 = r["out_x2"].transpose(1, 0, 2).reshape(1024, T)
        gkv = r["out_gkv"].transpose(1, 0, 2).reshape(256, T)
        rows = slice(g * 256, (g + 1) * 256)
        out[b, :, rows] = (x2[rows] + gkv).T
    return out


def kernel(**inputs):
    if "nc" not in _CACHE:
        _CACHE["nc"] = build_module()
    nc = _CACHE["nc"]
    in_maps = make_in_maps(inputs)
    from concourse.bass_utils import run_bass_kernel_spmd
    res = run_bass_kernel_spmd(nc, in_maps, list(range(NCORES))).results
    return assemble(res).astype(np.float32)


# revision 18
# speedup vs baseline: 1.0476x; 1.0476x over previous
"""Trainium2 Bass kernel for nn_Block_41893111005237 (Mamba2 + MQA + RWKV-CMix block).

Sharding: 2-way data-parallel over batch x 4-way tensor-parallel within each
group of 4 cores (mamba heads 8/core, attn q-heads 4/core with replicated KV,
FFN column/row split on W_key/W_val). Activations are feature-major [D, T]
on-chip. Large matmuls run as float32r (full PE rate at N=512); the SSD
chunked scan, attention probabilities, and the FFN use bf16 operands with
fp32 PSUM accumulation. ACT uses only exp/ln/abs/square/relu/identity (one
LUT table): softplus=ln(1+exp), sigmoid=exp(-ln(1+exp(-x))), rsqrt=exp(-.5 ln).
"""
import os
import sys
from contextlib import ExitStack

import numpy as np

for _p in ("/opt/trn_rl_repo", "/root/.axon_site/_ro/trn_rl_repo"):
    if os.path.isdir(_p) and _p not in sys.path:
        sys.path.insert(0, _p)

import ml_dtypes
import concourse.bass as bass
import concourse.tile as tile
from concourse import bacc, mybir
from concourse.bass import ts

f32 = mybir.dt.float32
f32r = mybir.dt.float32r
bf16 = mybir.dt.bfloat16
FT = mybir.ActivationFunctionType
OP = mybir.AluOpType

D = 1024
T = 1024
NCORES = 8
L = 128
NCH = 8
HPC = 8
P = 64
DI = 2048
AH = 4
HD = 64
EPS0 = 1e-6
EPS_G = 1e-5
GROUPS = [[0, 1, 2, 3], [4, 5, 6, 7]]

_CACHE = {}


def _patch_act_tables():
    # All ACT functions used here (exp/ln/copy/identity/square/abs/relu) live
    # in the natural_log_exp_and_others LUT set; restricting the chooser to it
    # avoids dozens of mid-kernel table reloads.
    import concourse.bacc as _bacc
    import concourse.hw_specs as _hw
    orig = _hw.get_activation_tables

    def only_lnexp(arch):
        t = orig(arch)
        if "natural_log_exp_and_others" not in t:
            return t
        return {k: (v if k == "natural_log_exp_and_others" else set())
                for k, v in t.items()}

    _bacc.get_activation_tables = only_lnexp


def build_module():
    _patch_act_tables()
    nc = bacc.Bacc("TRN2", target_bir_lowering=False, debug=False,
                   num_devices=NCORES)

    def din(name, shape, dt=f32r):
        return nc.dram_tensor(name, shape, dt, kind="ExternalInput").ap()

    xT_d = din("xT", [128, 8, T])
    wc_d = din("wc", [128, 8, 11, 128])
    wout_d = din("wout", [128, 4, 8, 128])
    wqkv_d = din("wqkv", [128, 8, 3, 128])
    wcp_d = din("wcp", [128, 2, 8, 128])
    wkey_d = din("wkey", [128, 8, 8, 128], bf16)
    wval_d = din("wval", [128, 8, 8, 128], bf16)
    wrec_d = din("wrec", [128, 8, 2, 128], bf16)
    cwm_d = din("cwm", [128, 6, 4], f32)
    cbm_d = din("cbm", [128, 6, 1], f32)
    cwa_d = din("cwa", [128, 3, 3], f32)
    cba_d = din("cba", [128, 3, 1], f32)
    acol_d = din("acol", [8, 1], f32)
    dtb_d = din("dtb", [8, 1], f32)
    dmrep_d = din("dmrep", [128, 4, 1], f32)
    maak_d = din("maak", [128, 8, 1], f32)
    maak1_d = din("maak1", [128, 8, 1], f32)
    maar_d = din("maar", [128, 8, 1], f32)
    maar1_d = din("maar1", [128, 8, 1], f32)
    idr_d = din("idr", [128, 128], f32r)
    idf_d = din("idf", [128, 128], f32)
    onesr_d = din("onesr", [128, 1], f32r)
    onesrow_d = din("onesrow", [1, 128], f32r)
    onesb_d = din("onesb", [128, 1], bf16)
    maskg_d = din("maskg", [128, 128], f32)
    maska_d = din("maska", [128, 896], bf16)

    out_gkv = nc.dram_tensor("out_gkv", [128, 2, T], f32,
                             kind="ExternalOutput").ap()
    out_x2 = nc.dram_tensor("out_x2", [128, 8, T], f32,
                            kind="ExternalOutput").ap()

    with tile.TileContext(nc) as tc, ExitStack() as ctx:
        outer = ctx.enter_context(tc.tile_pool(name="outer", bufs=1))
        scr = ctx.enter_context(tc.tile_pool(name="scr", bufs=2))
        wpool = ctx.enter_context(tc.tile_pool(name="wmt", bufs=4))
        dram = ctx.enter_context(tc.tile_pool(name="dram", bufs=1, space="DRAM"))

        def cinit(name, dram_ap, shape, dt=f32):
            t = outer.tile(shape, dt, tag=name)
            nc.sync.dma_start(t[:], dram_ap)
            return t

        idr = cinit("idr", idr_d, [128, 128], f32r)
        idf = cinit("idf", idf_d, [128, 128])
        onesr = cinit("onesr", onesr_d, [128, 1], f32r)
        onesrow = cinit("onesrow", onesrow_d, [1, 128], f32r)
        onesb = cinit("onesb", onesb_d, [128, 1], bf16)
        maskg = cinit("maskg", maskg_d, [128, 128])
        maska = cinit("maska", maska_d, [128, 896], bf16)
        cwm = cinit("cwm", cwm_d, [128, 6, 4])
        cbm = cinit("cbm", cbm_d, [128, 6, 1])
        cwa = cinit("cwa", cwa_d, [128, 3, 3])
        cba = cinit("cba", cba_d, [128, 3, 1])
        acol = cinit("acol", acol_d, [8, 1])
        dtb = cinit("dtb", dtb_d, [8, 1])
        dmrep = cinit("dmrep", dmrep_d, [128, 4, 1])
        maak = cinit("maak", maak_d, [128, 8, 1])
        maak1 = cinit("maak1", maak1_d, [128, 8, 1])
        maar = cinit("maar", maar_d, [128, 8, 1])
        maar1 = cinit("maar1", maar1_d, [128, 8, 1])
        epsrow = outer.tile([1, 1], f32, tag="epsrow")

        xres = outer.tile([128, 8, T], f32r, tag="xres")
        nc.sync.dma_start(xres[:], xT_d)

        def make_rs(get_kt, n_kt, den, eps_val, label, ps):
            acc = [ps.tile([1, 512], f32, tag="ssq", name=f"ssqa{i}") for i in range(2)]
            for kt in range(n_kt):
                for tb in range(2):
                    sq = scr.tile([128, 512], f32r, tag="sqws")
                    nc.scalar.activation(sq[:], get_kt(kt)[:, ts(tb, 512)],
                                         FT.Square)
                    nc.tensor.matmul(acc[tb][:], onesr[:], sq[:],
                                     start=(kt == 0), stop=(kt == n_kt - 1))
            nc.vector.memset(epsrow[:], eps_val)
            lnrow = scr.tile([1, T], f32, tag="lnrow", bufs=1)
            for tb in range(2):
                nc.scalar.activation(lnrow[:, ts(tb, 512)], acc[tb][:],
                                     FT.Ln, bias=epsrow[:], scale=1.0 / den)
            rsrow = scr.tile([1, T], f32r, tag="rsrow", bufs=1)
            nc.scalar.activation(rsrow[:], lnrow[:], FT.Exp, scale=-0.5)
            return rsrow

        def bcast_row(rsrow, ps):
            out_sb = scr.tile([128, T], f32, tag="rbX", bufs=1)
            for tb in range(2):
                pt = ps.tile([128, 512], f32, tag="mm")
                nc.tensor.matmul(pt[:], onesrow[:], rsrow[:, ts(tb, 512)],
                                 start=True, stop=True)
                nc.scalar.copy(out_sb[:, ts(tb, 512)], pt[:])
            return out_sb

        def psum_to_dram(pt_ap, dram_ap):
            stg = scr.tile([128, 512], f32, tag="stg", name="stg")
            nc.scalar.copy(stg[:], pt_ap)
            nc.sync.dma_start(dram_ap, stg[:])

        def sigmoid_into(out_ap, x_ap, pool, inplace_x=False):
            # out = sigmoid(x) = exp(-ln(1+exp(-x)))
            e = out_ap if inplace_x else pool.tile([128, x_ap.shape[-1]], f32,
                                                   tag="sigt")
            nc.scalar.activation(e[:], x_ap, FT.Exp, scale=-1.0)
            nc.vector.tensor_scalar_add(e[:], e[:], 1.0)
            nc.scalar.activation(e[:], e[:], FT.Ln)
            nc.scalar.activation(out_ap, e[:], FT.Exp, scale=-1.0)

        def silu_into(out_ap, x_ap, pool):
            s = pool.tile([128, x_ap.shape[-1]], f32, tag="silt")
            sigmoid_into(s[:], x_ap, pool, inplace_x=True)
            nc.vector.tensor_tensor(out_ap, x_ap, s[:], OP.mult)

        # ================= Stage M =================
        with tc.tile_pool(name="stM", bufs=1) as sm, \
             tc.tile_pool(name="stM2", bufs=2) as sm2, \
             tc.tile_pool(name="pmm", bufs=3, space="PSUM") as pmm, \
             tc.tile_pool(name="pssq", bufs=2, space="PSUM") as pssq, \
             tc.tile_pool(name="psp", bufs=3, space="PSUM") as psp:

            rs0 = make_rs(lambda kt: xres[:, kt, :].bitcast(f32), 8, D, EPS0,
                          "0", pssq)
            rs0b = bcast_row(rs0, pmm)

            xbcp = sm.tile([128, 6, T + 3], f32r, tag="conv")
            nc.vector.memset(xbcp[:, :, 0:3].bitcast(f32), 0.0)
            dtpre = sm.tile([8, T], f32, tag="dtpre")
            sz = sm.tile([128, 4, T], bf16, tag="sz")

            for mt in [10, 8, 9, 4, 5, 6, 7, 0, 1, 2, 3]:
                wt = wpool.tile([128, 8, 128], f32r, tag="wmt", bufs=3)
                nc.sync.dma_start(wt[:], wc_d[:, :, mt, :])
                for tb in range(2):
                    pt = pmm.tile([128, 512], f32, tag="mm")
                    for kt in range(8):
                        nc.tensor.matmul(pt[:], wt[:, kt, :],
                                         xres[:, kt, ts(tb, 512)],
                                         start=(kt == 0), stop=(kt == 7))
                    if mt == 10:
                        nc.vector.tensor_tensor(dtpre[:, ts(tb, 512)],
                                                pt[0:8, :], rs0b[0:8, ts(tb, 512)],
                                                OP.mult)
                    elif mt < 4:
                        zt = sm2.tile([128, 512], f32, tag="ztmp")
                        nc.vector.tensor_tensor(zt[:], pt[:], rs0b[:, ts(tb, 512)],
                                                OP.mult)
                        silu_into(sz[:, mt, ts(tb, 512)], zt[:], sm2)
                    else:
                        nc.vector.tensor_tensor(
                            xbcp[:, mt - 4, 3 + tb * 512:3 + (tb + 1) * 512],
                            pt[:], rs0b[:, ts(tb, 512)], OP.mult)

            for i in [4, 5, 0, 1, 2, 3]:
                tmpc = sm2.tile([128, T], f32, tag="convtmp")
                nc.scalar.activation(tmpc[:], xbcp[:, i, 0:T].bitcast(f32),
                                     FT.Identity,
                                     bias=cbm[:, i, :], scale=cwm[:, i, 0:1])
                for k in range(1, 4):
                    nc.vector.scalar_tensor_tensor(
                        tmpc[:], xbcp[:, i, k:k + T].bitcast(f32),
                        cwm[:, i, k:k + 1], tmpc[:], OP.mult, OP.add)
                silu_into(xbcp[:, i, 3:3 + T], tmpc[:], sm2)
            convo = xbcp[:, :, 3:3 + T]

            # dt -> softplus; chunk-local cumsums
            dtsp = sm.tile([8, T], f32, tag="dtsp")
            nc.scalar.activation(dtsp[:], dtpre[:], FT.Exp, bias=dtb[:])
            nc.vector.tensor_scalar_add(dtsp[:], dtsp[:], 1.0)
            nc.scalar.activation(dtsp[:], dtsp[:], FT.Ln)
            dtA = sm.tile([8, T], f32, tag="dtA")
            nc.vector.tensor_scalar_mul(dtA[:], dtsp[:], acol[:])
            zr8 = sm.tile([8, L], f32, tag="zr8")
            nc.vector.memset(zr8[:], 0.0)
            lcs = sm.tile([8, NCH, L], f32, tag="lcs")
            for c in range(NCH):
                nc.vector.tensor_tensor_scan(lcs[:, c, :], dtA[:, ts(c, L)],
                                             zr8[:], 0.0, OP.add, OP.add)
            lend0 = sm.tile([1, 8, 8], f32, tag="lend0")
            nc.sync.dma_start(lend0[:], lcs[:, :, L - 1])
            lts = sm.tile([128, 8, 8], f32, tag="lts")
            dtspT = sm.tile([128, 8, 8], f32, tag="dtspT")
            for c in range(NCH):
                ptr = psp.tile([128, 512], f32, tag="sp")
                nc.tensor.transpose(ptr[0:128, 0:8], lcs[:, c, :], idf[0:8, 0:8])
                nc.scalar.copy(lts[:, :, c], ptr[0:128, 0:8])
                ptr2 = psp.tile([128, 512], f32, tag="sp")
                nc.tensor.transpose(ptr2[0:128, 0:8], dtsp[:, ts(c, L)],
                                    idf[0:8, 0:8])
                nc.scalar.copy(dtspT[:, :, c], ptr2[0:128, 0:8])
            lrep = sm.tile([128, 8, 8], f32, tag="lrep")
            for h in range(HPC):
                nc.gpsimd.partition_broadcast(lrep[:, h, :], lend0[:, h, :])
            elrep = sm.tile([128, 8, 8], f32, tag="elrep")
            nc.scalar.activation(elrep[:], lrep[:], FT.Exp)
            fc = sm.tile([128, 8, 8], f32, tag="fc")
            nc.vector.tensor_tensor(fc[:], lrep[:], lts[:], OP.subtract)
            nc.scalar.activation(fc[:], fc[:], FT.Exp)
            nc.vector.tensor_tensor(fc[:], fc[:], dtspT[:], OP.mult)

            ym = sm.tile([128, 4, T], f32r, tag="ym")
            hst = [sm.tile([128, HPC, P], bf16, tag=f"hst{i}", name=f"hst{i}") for i in range(2)]
            nc.vector.memset(hst[0][:].bitcast(mybir.dt.uint16), 0)

            for c in range(NCH):
                csl = ts(c, L)
                gp = psp.tile([128, 512], f32, tag="sp")
                nc.tensor.matmul(gp[0:128, 0:128], convo[:, 4, csl],
                                 convo[:, 5, csl], start=True, stop=True)
                gm = sm2.tile([128, 128], f32, tag="gm")
                nc.vector.tensor_tensor(gm[:], gp[0:128, 0:128], maskg[:], OP.mult)
                btp = psp.tile([128, 512], f32r, tag="sp")
                nc.tensor.transpose(btp[0:128, 0:128], convo[:, 4, csl], idr[:])
                btm = sm2.tile([128, 128], bf16, tag="btm")
                nc.scalar.copy(btm[:], btp[0:128, 0:128].bitcast(f32))
                xtm = sm2.tile([128, HPC, P], bf16, tag="xtm")
                for pr in range(4):
                    xp = psp.tile([128, 512], f32r, tag="sp")
                    nc.tensor.transpose(xp[0:128, 0:128], convo[:, pr, csl], idr[:])
                    nc.scalar.copy(
                        xtm[:, pr * 2:pr * 2 + 2, :],
                        xp[0:128, 0:128].bitcast(f32)
                        .rearrange("p (a b) -> p a b", a=2))
                lcs0c = sm2.tile([1, 8, L], f32, tag="lcs0c")
                nc.sync.dma_start(lcs0c[:], lcs[:, c, :])
                lball = sm2.tile([128, HPC, L], f32, tag="lball", bufs=1)
                for h in range(HPC):
                    nc.gpsimd.partition_broadcast(lball[:, h, :],
                                                  lcs0c[:, h, :])
                mall = sm2.tile([128, HPC, L], f32, tag="mall", bufs=1)
                nc.vector.tensor_tensor(
                    mall[:], lball[:],
                    lts[:, :, c:c + 1].broadcast_to((128, 8, L)), OP.subtract)
                nc.vector.tensor_scalar_min(mall[:], mall[:], 0.0)
                nc.scalar.activation(mall[:], mall[:], FT.Exp)
                eall = sm2.tile([128, HPC, L], bf16, tag="eall")
                nc.scalar.activation(eall[:], lball[:], FT.Exp)
                sall = sm2.tile([128, HPC, L], bf16, tag="sall")
                nc.vector.tensor_tensor(
                    sall[:],
                    gm[:].rearrange("p (o t) -> p o t", o=1).broadcast_to((128, 8, L)),
                    mall[:], OP.mult)
                chat = sm2.tile([128, HPC, L], bf16, tag="chat")
                nc.vector.tensor_tensor(
                    chat[:],
                    convo[:, 5, csl].bitcast(f32)
                    .rearrange("p (o t) -> p o t", o=1)
                    .broadcast_to((128, 8, L)),
                    eall[:], OP.mult)
                dtx = sm2.tile([128, HPC, P], bf16, tag="dtx")
                nc.vector.tensor_tensor(
                    dtx[:], xtm[:],
                    dtspT[:, :, c:c + 1].broadcast_to((128, 8, P)), OP.mult)
                dtxd = sm2.tile([128, HPC, P], bf16, tag="dtxd")
                nc.vector.tensor_tensor(
                    dtxd[:], xtm[:],
                    fc[:, :, c:c + 1].broadcast_to((128, 8, P)), OP.mult)
                hprev, hnew = hst[c % 2], hst[(c + 1) % 2]
                updp = psp.tile([128, 512], f32, tag="sp")
                for h in range(HPC):
                    ypp = pmm.tile([128, 512], f32, tag="mm")
                    nc.tensor.matmul(ypp[0:P, 0:L], dtx[:, h, :], sall[:, h, :],
                                     start=True, stop=False)
                    nc.tensor.matmul(ypp[0:P, 0:L], hprev[:, h, :], chat[:, h, :],
                                     start=False, stop=True)
                    r0 = (h % 2) * P
                    nc.scalar.copy(ym[r0:r0 + P, h // 2, csl], ypp[0:P, 0:L])
                    nc.tensor.matmul(updp[:, ts(h, P)], btm[:], dtxd[:, h, :],
                                     start=True, stop=True)
                nc.vector.tensor_tensor(
                    hnew[:], hprev[:],
                    elrep[:, :, c:c + 1].broadcast_to((128, 8, P)), OP.mult)
                nc.vector.tensor_tensor(
                    hnew[:], hnew[:],
                    updp[:].rearrange("p (h q) -> p h q", h=8), OP.add)

            for i in range(4):
                nc.vector.scalar_tensor_tensor(
                    ym[:, i, :], convo[:, i, :].bitcast(f32), dmrep[:, i, :],
                    ym[:, i, :].bitcast(f32), OP.mult, OP.add)
                nc.vector.tensor_tensor(ym[:, i, :], ym[:, i, :].bitcast(f32),
                                        sz[:, i, :], OP.mult)

            sqacc = [pssq.tile([1, 512], f32, tag="ssq", name=f"ssqb{i}") for i in range(2)]
            for i in range(4):
                sq = scr.tile([128, T], f32r, tag="sqws")
                nc.scalar.activation(sq[:], ym[:, i, :].bitcast(f32), FT.Square)
                for tb in range(2):
                    nc.tensor.matmul(sqacc[tb][:], onesr[:], sq[:, ts(tb, 512)],
                                     start=(i == 0), stop=(i == 3))
            ar1_in = [dram.tile([1025, 512], f32, name=f"ar1i{tb}")
                      for tb in range(2)]
            ar1_out = [dram.tile([1025, 512], f32, name=f"ar1o{tb}")
                       for tb in range(2)]
            sqrow = scr.tile([1, T], f32, tag="lnrow", name="sqrow", bufs=1)
            for tb in range(2):
                nc.scalar.copy(sqrow[:, ts(tb, 512)], sqacc[tb][:])
                nc.sync.dma_start(ar1_in[tb][1024:1025, :],
                                  sqrow[:, ts(tb, 512)])
                for mt in range(8):
                    wo = wpool.tile([128, 8, 128], f32r, tag="wmt", bufs=3)
                    nc.sync.dma_start(wo[:, 0:4, :], wout_d[:, :, mt, :])
                    pt = pmm.tile([128, 512], f32, tag="mm")
                    for kt in range(4):
                        nc.tensor.matmul(pt[:], wo[:, kt, :],
                                         ym[:, kt, ts(tb, 512)],
                                         start=(kt == 0), stop=(kt == 3))
                    psum_to_dram(pt[:], ar1_in[tb][ts(mt, 128), :])
                nc.gpsimd.collective_compute(
                    "AllReduce", OP.add, replica_groups=GROUPS,
                    ins=[ar1_in[tb].opt()], outs=[ar1_out[tb].opt()])

            nc.vector.memset(epsrow[:], EPS_G)
            gs = scr.tile([1, T], f32, tag="lnrow", name="gs", bufs=1)
            rsg = scr.tile([1, T], f32r, tag="rsrow", name="rsg", bufs=1)
            for tb in range(2):
                nc.sync.dma_start(gs[:, ts(tb, 512)], ar1_out[tb][1024:1025, :])
                nc.scalar.activation(gs[:, ts(tb, 512)], gs[:, ts(tb, 512)],
                                     FT.Ln, bias=epsrow[:], scale=1.0 / DI)
                nc.scalar.activation(rsg[:, ts(tb, 512)], gs[:, ts(tb, 512)],
                                     FT.Exp, scale=-0.5)
            rsgb = bcast_row(rsg, pmm)
            for kt in range(8):
                for tb in range(2):
                    smt = scr.tile([128, 512], f32, tag="sumt", bufs=2)
                    nc.sync.dma_start(smt[:], ar1_out[tb][ts(kt, 128), :])
                    nc.vector.tensor_tensor(smt[:], smt[:],
                                            rsgb[:, ts(tb, 512)], OP.mult)
                    nc.vector.tensor_tensor(
                        xres[:, kt, ts(tb, 512)],
                        xres[:, kt, ts(tb, 512)].bitcast(f32), smt[:], OP.add)

        # ================= Stage A =================
        with tc.tile_pool(name="stA", bufs=1) as sa, \
             tc.tile_pool(name="stA2", bufs=2) as sa2, \
             tc.tile_pool(name="stA3", bufs=3) as sa3, \
             tc.tile_pool(name="pmm", bufs=3, space="PSUM") as pmm, \
             tc.tile_pool(name="pssq", bufs=2, space="PSUM") as pssq, \
             tc.tile_pool(name="psp", bufs=2, space="PSUM") as psp:

            rs1 = make_rs(lambda kt: xres[:, kt, :].bitcast(f32), 8, D, EPS0,
                          "1", pssq)
            rs1b = bcast_row(rs1, pmm)
            qkvs = sa.tile([128, 3, T + 2], f32, tag="qkvs")
            nc.vector.memset(qkvs[:, :, 0:2], 0.0)
            for mt in range(3):
                wq = wpool.tile([128, 8, 128], f32r, tag="wmt", bufs=3)
                nc.sync.dma_start(wq[:], wqkv_d[:, :, mt, :])
                for tb in range(2):
                    pt = pmm.tile([128, 512], f32, tag="mm")
                    for kt in range(8):
                        nc.tensor.matmul(pt[:], wq[:, kt, :],
                                         xres[:, kt, ts(tb, 512)],
                                         start=(kt == 0), stop=(kt == 7))
                    nc.vector.tensor_tensor(
                        qkvs[:, mt, 2 + tb * 512:2 + (tb + 1) * 512], pt[:],
                        rs1b[:, ts(tb, 512)], OP.mult)
            convA = sa.tile([128, 3, T], f32r, tag="convA")
            for i in range(3):
                tmpc = sa2.tile([128, T], f32, tag="convtA")
                nc.scalar.activation(tmpc[:], qkvs[:, i, 0:T], FT.Identity,
                                     bias=cba[:, i, :], scale=cwa[:, i, 0:1])
                for k in range(1, 3):
                    nc.vector.scalar_tensor_tensor(
                        tmpc[:], qkvs[:, i, k:k + T], cwa[:, i, k:k + 1],
                        tmpc[:], OP.mult, OP.add)
                nc.vector.tensor_copy(convA[:, i, :], tmpc[:])
            ka2 = sa.tile([128, T], f32r, tag="ka2")
            nc.sync.dma_start(ka2[0:64, :], convA[0:64, 2, :])
            nc.sync.dma_start(ka2[64:128, :], convA[0:64, 2, :])
            vtm = sa.tile([128, NCH, HD + 1], bf16, tag="vtm")
            nc.vector.tensor_copy(
                vtm[:, :, HD:HD + 1],
                onesb[:].rearrange("p (a o) -> p a o", a=1).broadcast_to((128, 8, 1)))
            for tk in range(NCH):
                vp = psp.tile([128, 512], f32r, tag="sp")
                nc.tensor.transpose(vp[0:128, 0:HD], convA[64:128, 2, ts(tk, L)],
                                    idr[64:128, 64:128])
                nc.scalar.copy(vtm[:, tk, 0:HD], vp[0:128, 0:HD].bitcast(f32))
            yat = sa.tile([128, 2, T], f32r, tag="yat")
            for h in range(AH):
                q0 = (h % 2) * 64
                for tb in range(2):
                    ypp = pmm.tile([128, 512], f32, tag="mm")
                    ntk = 4 * (tb + 1)
                    for tk in range(ntk):
                        sp = pmm.tile([128, 512], f32, tag="mm")
                        nc.tensor.matmul(
                            sp[:], ka2[q0:q0 + 64, ts(tk, L)],
                            convA[q0:q0 + 64, h // 2, ts(tb, 512)],
                            start=True, stop=True)
                        ptile = sa3.tile([128, 512], bf16, tag="ptile")
                        nc.scalar.activation(ptile[:], sp[:], FT.Exp, scale=0.125)
                        delta = tb * 512 - tk * 128
                        if delta < 127:
                            nc.vector.tensor_tensor(
                                ptile[:], ptile[:],
                                maska[:, 384 + delta:896 + delta], OP.mult)
                        nc.tensor.matmul(ypp[0:HD + 1, :], vtm[:, tk, :],
                                         ptile[:], start=(tk == 0),
                                         stop=(tk == ntk - 1))
                    denr = sa2.tile([1, 512], f32, tag="denr")
                    nc.scalar.activation(denr[:], ypp[HD:HD + 1, :], FT.Ln)
                    rd = sa2.tile([1, 512], f32r, tag="rd")
                    nc.scalar.activation(rd[:], denr[:], FT.Exp, scale=-1.0)
                    rdp = psp.tile([128, 512], f32, tag="sp")
                    nc.tensor.matmul(rdp[0:64, :], onesrow[:, 0:64], rd[:],
                                     start=True, stop=True)
                    rdb = sa2.tile([64, 512], f32, tag="rdb")
                    nc.scalar.copy(rdb[:], rdp[0:64, :])
                    nc.vector.tensor_tensor(
                        yat[q0:q0 + 64, h // 2, ts(tb, 512)],
                        ypp[0:HD, :], rdb[:], OP.mult)
            ar2_in = [dram.tile([1024, 512], f32, name=f"ar2i{tb}")
                      for tb in range(2)]
            ar2_out = [dram.tile([1024, 512], f32, name=f"ar2o{tb}")
                       for tb in range(2)]
            for tb in range(2):
                for mt in range(8):
                    wcpt = wpool.tile([128, 8, 128], f32r, tag="wmt", bufs=3)
                    nc.sync.dma_start(wcpt[:, 0:2, :], wcp_d[:, :, mt, :])
                    pt = pmm.tile([128, 512], f32, tag="mm")
                    for kt in range(2):
                        nc.tensor.matmul(pt[:], wcpt[:, kt, :],
                                         yat[:, kt, ts(tb, 512)],
                                         start=(kt == 0), stop=(kt == 1))
                    psum_to_dram(pt[:], ar2_in[tb][ts(mt, 128), :])
                nc.gpsimd.collective_compute(
                    "AllReduce", OP.add, replica_groups=GROUPS,
                    ins=[ar2_in[tb].opt()], outs=[ar2_out[tb].opt()])
            for kt in range(8):
                for tb in range(2):
                    smt = scr.tile([128, 512], f32, tag="sumt", bufs=2)
                    nc.sync.dma_start(smt[:], ar2_out[tb][ts(kt, 128), :])
                    nc.vector.tensor_tensor(
                        xres[:, kt, ts(tb, 512)],
                        xres[:, kt, ts(tb, 512)].bitcast(f32), smt[:], OP.add)
            nc.sync.dma_start(out_x2, xres[:].bitcast(f32))

        # ================= Stage F =================
        with tc.tile_pool(name="stF", bufs=1) as sf, \
             tc.tile_pool(name="stF2", bufs=2) as sf2, \
             tc.tile_pool(name="pmm", bufs=3, space="PSUM") as pmm, \
             tc.tile_pool(name="pssq", bufs=2, space="PSUM") as pssq:

            rs2 = make_rs(lambda kt: xres[:, kt, :].bitcast(f32), 8, D, EPS0,
                          "2", pssq)
            rs2b = bcast_row(rs2, pmm)
            h2 = sf.tile([128, 8, T + 1], f32, tag="h2")
            nc.vector.memset(h2[:, :, 0:1], 0.0)
            for kt in range(8):
                nc.vector.tensor_tensor(h2[:, kt, 1:T + 1],
                                        xres[:, kt, :].bitcast(f32), rs2b[:],
                                        OP.mult)
            xk = sf.tile([128, 8, T], bf16, tag="xk")
            xr = sf.tile([128, 8, T], bf16, tag="xr")
            for kt in range(8):
                xsc = sf2.tile([128, T], f32, tag="xscr")
                nc.vector.tensor_scalar_mul(xsc[:], h2[:, kt, 1:T + 1],
                                            maak1[:, kt, :])
                nc.vector.scalar_tensor_tensor(
                    xk[:, kt, :], h2[:, kt, 0:T], maak[:, kt, :], xsc[:],
                    OP.mult, OP.add)
                xsc2 = sf2.tile([128, T], f32, tag="xscr")
                nc.vector.tensor_scalar_mul(xsc2[:], h2[:, kt, 1:T + 1],
                                            maar1[:, kt, :])
                nc.vector.scalar_tensor_tensor(
                    xr[:, kt, :], h2[:, kt, 0:T], maar[:, kt, :], xsc2[:],
                    OP.mult, OP.add)
            kf = sf.tile([128, 8, T], bf16, tag="kf")
            for mt in range(8):
                wk = wpool.tile([128, 8, 128], bf16, tag="wmtb", bufs=2)
                nc.sync.dma_start(wk[:], wkey_d[:, :, mt, :])
                for tb in range(2):
                    pt = pmm.tile([128, 512], f32, tag="mm")
                    for kt in range(8):
                        nc.tensor.matmul(pt[:], wk[:, kt, :],
                                         xk[:, kt, ts(tb, 512)],
                                         start=(kt == 0), stop=(kt == 7))
                    rl = sf2.tile([128, 512], f32, tag="rl")
                    nc.scalar.activation(rl[:], pt[:], FT.Relu)
                    nc.scalar.activation(kf[:, mt, ts(tb, 512)], rl[:],
                                         FT.Square)
            rsc_in = [dram.tile([1024, 512], f32, name=f"rsci{tb}")
                      for tb in range(2)]
            rsc_out = [dram.tile([256, 512], f32, name=f"rsco{tb}")
                       for tb in range(2)]
            for tb in range(2):
                for mt in range(8):
                    wv = wpool.tile([128, 8, 128], bf16, tag="wmtb", bufs=2)
                    nc.sync.dma_start(wv[:], wval_d[:, :, mt, :])
                    pt = pmm.tile([128, 512], f32, tag="mm")
                    for kt in range(8):
                        nc.tensor.matmul(pt[:], wv[:, kt, :],
                                         kf[:, kt, ts(tb, 512)],
                                         start=(kt == 0), stop=(kt == 7))
                    psum_to_dram(pt[:], rsc_in[tb][ts(mt, 128), :])
                nc.gpsimd.collective_compute(
                    "ReduceScatter", OP.add, replica_groups=GROUPS,
                    ins=[rsc_in[tb].opt()], outs=[rsc_out[tb].opt()])
            kvr = sf.tile([128, 2, T], f32, tag="kvr")
            for tb in range(2):
                nc.sync.dma_start(
                    kvr[:, :, ts(tb, 512)],
                    rsc_out[tb][:].rearrange("(k p) t -> p k t", p=128))
            sg = sf.tile([128, 2, T], f32, tag="sg")
            for mt in range(2):
                wr = wpool.tile([128, 8, 128], bf16, tag="wmtb", bufs=2)
                nc.sync.dma_start(wr[:], wrec_d[:, :, mt, :])
                for tb in range(2):
                    pt = pmm.tile([128, 512], f32, tag="mm")
                    for kt in range(8):
                        nc.tensor.matmul(pt[:], wr[:, kt, :],
                                         xr[:, kt, ts(tb, 512)],
                                         start=(kt == 0), stop=(kt == 7))
                    e_ = sf2.tile([128, 512], f32, tag="sige")
                    nc.scalar.activation(e_[:], pt[:], FT.Exp, scale=-1.0)
                    nc.vector.tensor_scalar_add(e_[:], e_[:], 1.0)
                    nc.scalar.activation(e_[:], e_[:], FT.Ln)
                    nc.scalar.activation(sg[:, mt, ts(tb, 512)], e_[:], FT.Exp,
                                         scale=-1.0)
            nc.vector.tensor_tensor(sg[:], sg[:], kvr[:], OP.mult)
            nc.sync.dma_start(out_gkv, sg[:])

    nc.compile()
    return nc


def _w_tiles(w, kt, mt, dt=np.float32):
    Dk_, Mm_ = kt * 128, mt * 128
    assert w.shape == (Dk_, Mm_), (w.shape, kt, mt)
    return np.ascontiguousarray(
        w.reshape(kt, 128, mt, 128).transpose(1, 0, 2, 3)).astype(dt)


def make_in_maps(inputs):
    f = lambda k: np.asarray(inputs[k], np.float32)
    x = f("x")
    W_in = f("W_in"); conv_w = f("conv_w"); conv_b = f("conv_b")
    A = -np.exp(f("A_log")); Dm = f("Dm"); dtbv = f("dt_bias")
    W_out = f("W_out") * f("mnorm_w")[:, None]
    W_qkv = f("W_qkv"); W_cproj = f("W_cproj")
    qw, qb = f("qconv_w"), f("qconv_b")
    kw, kb = f("kconv_w"), f("kconv_b")
    vw, vb = f("vconv_w"), f("vconv_b")
    maa_k = f("time_maa_k"); maa_r = f("time_maa_r")
    W_key = f("W_key"); W_rec = f("W_rec"); W_val = f("W_val")
    bfdt = ml_dtypes.bfloat16

    idm = np.eye(128, dtype=np.float32)
    maskg = (np.arange(128)[:, None] <= np.arange(128)[None, :]).astype(np.float32)
    cgrid = np.arange(896)[None, :] - 384
    maska = (np.arange(128)[:, None] <= cgrid).astype(bfdt)
    cwa_full = np.concatenate([qw, qw, qw, qw, kw, vw], 0)       # (384, 3)
    cba_full = np.concatenate([qb, qb, qb, qb, kb, vb], 0)

    in_maps = []
    for core in range(NCORES):
        b, g = core // 4, core % 4
        zc = W_in[:, g * 512:(g + 1) * 512]
        xc = W_in[:, 2048 + g * 512:2048 + (g + 1) * 512]
        Bc = W_in[:, 4096:4224]; Cc = W_in[:, 4224:4352]
        dc = W_in[:, 4352 + g * 8:4352 + (g + 1) * 8]
        dpad = np.zeros((D, 120), np.float32)
        W_core = np.concatenate([zc, xc, Bc, Cc, dc, dpad], 1)
        cw = np.concatenate([conv_w[g * 512:(g + 1) * 512], conv_w[2048:2304]], 0)
        cb = np.concatenate([conv_b[g * 512:(g + 1) * 512], conv_b[2048:2304]], 0)
        Wq_c = np.concatenate([W_qkv[:, g * 256:(g + 1) * 256],
                               W_qkv[:, 1024:1152]], 1)
        m = {
            "xT": np.ascontiguousarray(
                x[b].T.reshape(8, 128, T).transpose(1, 0, 2)),
            "wc": _w_tiles(W_core, 8, 11),
            "wout": _w_tiles(W_out[g * 512:(g + 1) * 512], 4, 8),
            "wqkv": _w_tiles(Wq_c, 8, 3),
            "wcp": _w_tiles(W_cproj[g * 256:(g + 1) * 256], 2, 8),
            "wkey": _w_tiles(W_key[:, g * 1024:(g + 1) * 1024], 8, 8, bfdt),
            "wval": _w_tiles(W_val[g * 1024:(g + 1) * 1024], 8, 8, bfdt),
            "wrec": _w_tiles(W_rec[:, g * 256:(g + 1) * 256], 8, 2, bfdt),
            "cwm": np.ascontiguousarray(cw.reshape(6, 128, 4).transpose(1, 0, 2)),
            "cbm": np.ascontiguousarray(cb.reshape(6, 128, 1).transpose(1, 0, 2)),
            "cwa": np.ascontiguousarray(
                cwa_full.reshape(3, 128, 3).transpose(1, 0, 2)),
            "cba": np.ascontiguousarray(
                cba_full.reshape(3, 128, 1).transpose(1, 0, 2)),
            "acol": A[g * 8:(g + 1) * 8, None],
            "dtb": dtbv[g * 8:(g + 1) * 8, None],
            "dmrep": np.ascontiguousarray(
                np.repeat(Dm[g * 8:(g + 1) * 8], 64)
                .reshape(4, 128, 1).transpose(1, 0, 2)),
            "maak": np.ascontiguousarray(
                maa_k.reshape(8, 128, 1).transpose(1, 0, 2)),
            "maak1": np.ascontiguousarray(
                (1.0 - maa_k).reshape(8, 128, 1).transpose(1, 0, 2)),
            "maar": np.ascontiguousarray(
                maa_r.reshape(8, 128, 1).transpose(1, 0, 2)),
            "maar1": np.ascontiguousarray(
                (1.0 - maa_r).reshape(8, 128, 1).transpose(1, 0, 2)),
            "idr": idm, "idf": idm,
            "onesr": np.ones((128, 1), np.float32),
            "onesrow": np.ones((1, 128), np.float32),
            "onesb": np.ones((128, 1), bfdt),
            "maskg": maskg, "maska": maska,
        }
        out = {}
        for k, v in m.items():
            if v.dtype == bfdt:
                out[k] = np.ascontiguousarray(v)
            else:
                out[k] = np.ascontiguousarray(v, np.float32)
        in_maps.append(out)
    return in_maps


def assemble(results):
    out = np.zeros((2, T, D), np.float32)
    for core in range(NCORES):
        b, g = core // 4, core % 4
        r = results[core]
        x2 = r["out_x2"].transpose(1, 0, 2).reshape(1024, T)
        gkv = r["out_gkv"].transpose(1, 0, 2).reshape(256, T)
        rows = slice(g * 256, (g + 1) * 256)
        out[b, :, rows] = (x2[rows] + gkv).T
    return out


def kernel(**inputs):
    if "nc" not in _CACHE:
        _CACHE["nc"] = build_module()
    nc = _CACHE["nc"]
    in_maps = make_in_maps(inputs)
    from concourse.bass_utils import run_bass_kernel_spmd
    res = run_bass_kernel_spmd(nc, in_maps, list(range(NCORES))).results
    return assemble(res).astype(np.float32)
